# revision 6
# baseline (speedup 1.0000x reference)
"""Trainium2 Bass kernel for nn_BagModel_3d (segment_reduce).

Computation (per bag b):
  out[b] = (1/n_b) * sum_{i < n_b} relu(x[b, i, :] @ W1 + b1) @ W2 + b2

Strategy (v2 -- instances-on-partitions dataflow):

- Data-parallel over bags: LPT assigns exactly 32 bags per core. Each
  core's valid instance columns are laid out contiguously (bag-major) and
  split into 128-column CHUNKs; x is cast to bf16 ON HOST so the dominant
  DMA moves half the bytes of the f32 baseline.
- Per chunk, the x block [2x128 din, 128 cols] is the matmul STATIONARY
  operand and W1 streams, so z lands as [cols(part), dh(free)] in PSUM.
  Two chunks share one PSUM bank; one relu instruction per bank
  ([128, 512], ScalarE activation / VectorE tensor_scalar alternating)
  produces h in bf16 -- no per-bag drains, no on-device casts.
- The ragged per-bag sum is a 0/1 selector matmul: praw[bag, dh] +=
  sel_c^T @ h_c, accumulated in one PSUM tile across all chunks. Bag
  boundaries are data (sel), so bags span chunks freely and padding
  columns contribute exactly 0. With COL_TILE, chunks are striped over
  the 4 PE column-groups (tile_position=(0,32j)) so 4 selector matmuls
  run concurrently in the array.
- Finalization: o4 = sum_dh praw*W2 (one fused DVE scalar_tensor_tensor
  with accum), strip-sum via a tiny matmul, then out = o4/n + b2.
- b1 (zero in this workload) is handled generally via a rank-1 k=1
  matmul (ones x b1row) added into z before relu, emitted only when
  b1 != 0 (program is keyed on that).
"""
import sys
import numpy as np

sys.path.insert(0, '/opt/trn_rl_repo')

import ml_dtypes

B, N_MAX, D_IN, D_H = 256, 512, 256, 256
N_CORES = 8
BAGS = B // N_CORES          # 32 bags per core
CHUNK = 128                  # instance columns per matmul chunk
COL_TILE = True              # stripe selector matmuls over PE col-groups
SEL_LAG = 2                  # pairs of lookahead before emitting sel MMs


def _piece_sched(nch):
    """Ramped x-DMA piece sizes (in chunks), round-robin over the three
    DMA issue rings (sync/scalar HWDGE + gpsimd SWDGE). Small pieces
    first so the PE can start early; big pieces later for bandwidth."""
    sizes = []
    ramp = [4, 4, 8, 8, 12, 12]
    left = nch
    for s in ramp:
        if left <= 0:
            break
        s = min(s, left)
        sizes.append(s)
        left -= s
    while left > 0:
        s = min(16, left)
        sizes.append(s)
        left -= s
    out = []
    c0 = 0
    for i, s in enumerate(sizes):
        out.append((c0, s, i % 3))
        c0 += s
    return out

_PROGRAM = None
_PROGRAM_KEY = None
_PLAN = None


def _make_plan(n, b2_value, has_b1):
    n = np.asarray(n, dtype=np.int64)
    order = np.argsort(-n, kind="stable")
    loads = [0] * N_CORES
    assign = [[] for _ in range(N_CORES)]
    for b in order:
        cands = [i for i in range(N_CORES) if len(assign[i]) < BAGS]
        c = min(cands, key=lambda i: (loads[i], len(assign[i]), i))
        assign[c].append(int(b))
        loads[c] += int(n[b])
    nch = (max(loads) + CHUNK - 1) // CHUNK
    nch += nch % 2                      # even: two chunks per PSUM bank
    return {
        "assign": assign,
        "loads": loads,
        "nch": int(nch),
        "n": [int(v) for v in n],
        "b2": float(b2_value),
        "has_b1": bool(has_b1),
    }


def _build_program(plan):
    import concourse.bacc as bacc
    import concourse.tile as tile
    from concourse import mybir

    f32 = mybir.dt.float32
    bf16 = mybir.dt.bfloat16
    i32 = mybir.dt.int32
    Alu = mybir.AluOpType
    Act = mybir.ActivationFunctionType

    NCH = plan["nch"]
    NPAIR = NCH // 2
    has_b1 = plan["has_b1"]
    pieces = _piece_sched(NCH)

    nc = bacc.Bacc("TRN2", target_bir_lowering=False, debug=False)

    xall = nc.dram_tensor("xall", [128, NCH * 256], bf16, kind="ExternalInput").ap()
    selm = nc.dram_tensor("selm", [128, NCH * BAGS], bf16, kind="ExternalInput").ap()
    n_col = nc.dram_tensor("n_col", [BAGS, 1], i32, kind="ExternalInput").ap()
    w1a = nc.dram_tensor("w1a", [128, D_H], bf16, kind="ExternalInput").ap()
    w1b = nc.dram_tensor("w1b", [128, D_H], bf16, kind="ExternalInput").ap()
    w2rep = nc.dram_tensor("w2rep", [128, D_H], f32, kind="ExternalInput").ap()
    stripm = nc.dram_tensor("stripm", [128, BAGS], f32, kind="ExternalInput").ap()
    if has_b1:
        b1row = nc.dram_tensor("b1row", [1, D_H], bf16, kind="ExternalInput").ap()
    out = nc.dram_tensor("out", [BAGS, 1], f32, kind="ExternalOutput").ap()

    rings = [nc.sync, nc.scalar, nc.gpsimd]
    with tile.TileContext(nc) as tc:
        with (
            tc.tile_pool(name="const", bufs=1) as cpool,
            tc.tile_pool(name="h", bufs=10) as hpool,
            tc.tile_pool(name="z", bufs=6, space="PSUM") as zpool,
            tc.tile_pool(name="pr", bufs=1, space="PSUM") as ppool,
        ):
            # ---- ring heads: weights on sync, selector on scalar,
            # small consts on gpsimd; then x pieces round-robin over all
            # three rings (each ring serializes its ops, so spreading
            # pieces triples DMA issue throughput) ----
            w1at = cpool.tile([128, D_H], bf16, tag="w1a")
            nc.sync.dma_start(w1at[:], w1a[:])
            w1bt = cpool.tile([128, D_H], bf16, tag="w1b")
            nc.sync.dma_start(w1bt[:], w1b[:])

            selt = cpool.tile([128, NCH * BAGS], bf16, tag="selt")
            nc.scalar.dma_start(selt[:], selm[:])

            w2t = cpool.tile([128, D_H], f32, tag="w2t")
            nc.gpsimd.dma_start(w2t[:], w2rep[:])
            stript = cpool.tile([128, BAGS], f32, tag="stript")
            nc.gpsimd.dma_start(stript[:], stripm[:])
            nI = cpool.tile([BAGS, 1], i32, tag="nI")
            nc.gpsimd.dma_start(nI[:], n_col[:])
            if has_b1:
                b1t = cpool.tile([1, D_H], bf16, tag="b1t")
                nc.gpsimd.dma_start(b1t[:], b1row[:])
                ones1 = cpool.tile([1, 128], bf16, tag="ones1")
                nc.vector.memset(ones1[:], 1.0)

            xsb = [None] * NCH          # chunk -> (tile, col offset)
            for (c0, sz, ring) in pieces:
                t = cpool.tile([128, sz * 256], bf16, tag=f"xsb{c0}")
                rings[ring].dma_start(t[:], xall[:, c0 * 256:(c0 + sz) * 256])
                for c in range(c0, c0 + sz):
                    xsb[c] = (t, (c - c0) * 256)

            def x_ap(c, half):
                t, off = xsb[c]
                return t[:, off + 128 * half:off + 128 * half + 128]

            # n-derived scalars (vector, overlapped with DMA fill)
            nf = cpool.tile([BAGS, 1], f32, tag="nf")
            nc.vector.tensor_copy(nf[:], nI[:])
            inv = cpool.tile([BAGS, 1], f32, tag="inv")
            nc.vector.reciprocal(inv[:], nf[:])

            # per-bag raw sums: 4 col-group strips (or strip 0 only)
            praw = ppool.tile([128, D_H], f32, tag="praw",
                              padded_shape=[128, 512])

            # selector MM bookkeeping: chunk c -> strip c%4 (COL_TILE)
            nstrip = 4 if COL_TILE else 1
            strip_chunks = [[c for c in range(NCH) if c % nstrip == j]
                            for j in range(nstrip)]
            first_c = {ch[0] for ch in strip_chunks if ch}
            last_c = {ch[-1] for ch in strip_chunks if ch}

            h_t = [None] * NPAIR

            def emit_pair(p):
                zp = zpool.tile([128, 512], f32, tag="z", name=f"z_{p}")
                for half in (0, 1):
                    c = 2 * p + half
                    zc = zp[:, 256 * half:256 * half + 256]
                    nc.tensor.matmul(zc, x_ap(c, 0), w1at[:],
                                     start=True, stop=False)
                    nc.tensor.matmul(zc, x_ap(c, 1), w1bt[:],
                                     start=False, stop=not has_b1)
                    if has_b1:
                        nc.tensor.matmul(zc, ones1[:], b1t[:],
                                         start=False, stop=True)
                hp = hpool.tile([128, 512], bf16, tag="h", name=f"h_{p}")
                if p % 2 == 0:
                    nc.scalar.activation(hp[:], zp[:], Act.Relu)
                else:
                    nc.vector.tensor_scalar(hp[:], zp[:], 0.0, None,
                                            op0=Alu.max)
                h_t[p] = hp

            def emit_sel_batch(bi):
                for c in range(4 * bi, min(4 * bi + 4, NCH)):
                    j = c % nstrip
                    hp = h_t[c // 2]
                    rhs = hp[:, 256 * (c % 2):256 * (c % 2) + 256]
                    tp = (0, 32 * j) if COL_TILE else None
                    nc.tensor.matmul(praw[32 * j:32 * j + 32, :],
                                     selt[:, c * BAGS:(c + 1) * BAGS], rhs,
                                     start=(c in first_c), stop=(c in last_c),
                                     tile_position=tp, skip_group_check=True)

            nbatch = (NCH + 3) // 4
            done_b = 0
            for p in range(NPAIR):
                emit_pair(p)
                b = (p - SEL_LAG - 1) // 2
                while done_b <= b:
                    emit_sel_batch(done_b)
                    done_b += 1
            while done_b < nbatch:
                emit_sel_batch(done_b)
                done_b += 1

            # ---- finalization ----
            stt = cpool.tile([128, D_H], f32, tag="stt")
            o4 = cpool.tile([128, 1], f32, tag="o4")
            if COL_TILE:
                nc.vector.scalar_tensor_tensor(
                    stt[:], praw[:], 1.0, w2t[:],
                    op0=Alu.mult, op1=Alu.mult, accum_out=o4[:])
                po = zpool.tile([BAGS, 1], f32, tag="z", name="po",
                                padded_shape=[128, 512])
                nc.tensor.matmul(po[:], stript[:], o4[:], start=True, stop=True)
                osb = cpool.tile([BAGS, 1], f32, tag="osb")
                nc.vector.tensor_scalar(osb[:], po[:], inv[:],
                                        float(plan["b2"]),
                                        op0=Alu.mult, op1=Alu.add)
            else:
                nc.vector.scalar_tensor_tensor(
                    stt[0:BAGS, :], praw[0:BAGS, :], 1.0, w2t[0:BAGS, :],
                    op0=Alu.mult, op1=Alu.mult, accum_out=o4[0:BAGS, :])
                osb = cpool.tile([BAGS, 1], f32, tag="osb")
                nc.vector.tensor_scalar(osb[:], o4[0:BAGS, :], inv[:],
                                        float(plan["b2"]),
                                        op0=Alu.mult, op1=Alu.add)
            nc.sync.dma_start(out[:], osb[:])

    nc.compile()
    return nc


def get_program(plan):
    global _PROGRAM, _PROGRAM_KEY
    key = (plan["b2"], plan["nch"], plan["has_b1"], COL_TILE)
    if _PROGRAM is None or _PROGRAM_KEY != key:
        _PROGRAM = _build_program(plan)
        _PROGRAM_KEY = key
    return _PROGRAM


def make_in_maps(x, n_instances, W1, b1, W2, b2=None):
    global _PLAN
    x = np.asarray(x, dtype=np.float32)
    n = np.asarray(n_instances, dtype=np.int32)
    W1 = np.asarray(W1, dtype=np.float32)
    b1 = np.asarray(b1, dtype=np.float32).reshape(-1)
    W2 = np.asarray(W2, dtype=np.float32).reshape(-1)
    b2v = 0.0 if b2 is None else float(np.asarray(b2).reshape(-1)[0])
    has_b1 = bool(np.any(b1 != 0.0))
    plan = _make_plan(n, b2v, has_b1)
    _PLAN = plan
    assign, NCH = plan["assign"], plan["nch"]
    COLS = NCH * CHUNK

    w1a = np.ascontiguousarray(W1[0:128, :]).astype(ml_dtypes.bfloat16)
    w1b = np.ascontiguousarray(W1[128:256, :]).astype(ml_dtypes.bfloat16)
    w2rep = np.ascontiguousarray(
        np.broadcast_to(W2.reshape(1, D_H), (128, D_H))).astype(np.float32)
    stripm = np.zeros((128, BAGS), dtype=np.float32)
    for j in range(4 if COL_TILE else 1):
        stripm[32 * j + np.arange(32), np.arange(32)] = 1.0

    in_maps = []
    for c in range(N_CORES):
        bags = assign[c]
        ns = np.array([n[b] for b in bags], dtype=np.int64)
        starts = np.concatenate([[0], np.cumsum(ns)])
        total = int(starts[-1])
        # X: [256 din, COLS] valid instance columns, bag-major
        X = np.zeros((D_IN, COLS), dtype=np.float32)
        for s, b in enumerate(bags):
            X[:, starts[s]:starts[s + 1]] = x[b, :ns[s], :].T
        # chunk-major xall: [128, (chunk, half, col)]
        Xr = X.reshape(D_IN, NCH, CHUNK)
        xa = np.empty((128, NCH, 256), dtype=np.float32)
        xa[:, :, 0:128] = Xr[0:128]
        xa[:, :, 128:256] = Xr[128:256]
        # selector: sel[(col %128), chunk*BAGS + slot] = 1 for valid cols
        sel = np.zeros((128, NCH, BAGS), dtype=np.float32)
        cols_idx = np.arange(total)
        slot_of = np.repeat(np.arange(BAGS), ns)
        sel[cols_idx % CHUNK, cols_idx // CHUNK, slot_of] = 1.0
        im = {
            "xall": xa.reshape(128, NCH * 256).astype(ml_dtypes.bfloat16),
            "selm": sel.reshape(128, NCH * BAGS).astype(ml_dtypes.bfloat16),
            "n_col": np.ascontiguousarray(
                ns.astype(np.int32).reshape(BAGS, 1)),
            "w1a": w1a, "w1b": w1b, "w2rep": w2rep, "stripm": stripm,
        }
        if has_b1:
            im["b1row"] = b1.reshape(1, D_H).astype(ml_dtypes.bfloat16)
        in_maps.append(im)
    return in_maps


def run_spmd(in_maps, b2_value=0.0, trace=False, **kwargs):
    from concourse import bass_utils
    if trace:
        # no S3 in this environment; keep trace artifacts local
        bass_utils.upload_artifacts = lambda tmpdir: tmpdir
    nc = get_program(_PLAN)
    return bass_utils.run_bass_kernel_spmd(
        nc, in_maps, core_ids=list(range(N_CORES)), trace=trace, **kwargs)


def kernel(x, n_instances, W1, b1, W2, b2):
    b2_value = float(np.asarray(b2).reshape(-1)[0])
    in_maps = make_in_maps(x, n_instances, W1, b1, W2, b2)
    res = run_spmd(in_maps, b2_value=b2_value)
    out = np.empty((B, 1), dtype=np.float32)
    for c in range(N_CORES):
        vals = res.results[c]["out"]
        for s, b in enumerate(_PLAN["assign"][c]):
            out[b, 0] = vals[s, 0]
    return out
